# revision 16
# baseline (speedup 1.0000x reference)
"""Deformable-ROI bilinear feature gather (KeypPointBboxNet) on 8 TRN2 cores.

v3: feat_map sharded on batch (one image per core), stored as FOUR
parity-shifted 2x2-pixel-block layouts in fp16: replica (ph,pw), block
(h2,w2) holds pixels (2*h2+ph+{0,1}, 2*w2+pw+{0,1}) x 256ch = 2KB
contiguous. A bilinear sample's 4 taps always live in exactly one block
of the replica matching (hl&1, wl&1), so each point is ONE 2KB gather
descriptor (full DMA-bus rate; half the SWDGE descriptor load of a
row-pair scheme).

Per core:
  - coordinate/weight pipeline in [128, S] f32 on DVE,
  - block index = (2*ph+pw)*4096 + (hl>>1)*64 + (wl>>1), shuffled to the
    16-partition-wrapped gather layout via 8 small DMAs + 3 replications,
  - combine alternates PE path (4 accumulating diag(w_k) @ v_k matmuls
    into PSUM, diags prebuilt once on DVE, ACT evacuates) with a DVE/ACT
    fp16 path, per slot,
  - per-chunk fp16 stores overlap the next chunk's gather.
"""

import math

import numpy as np

B, C, H, W = 8, 256, 128, 128
N_ROIS, NUM_POINT, STRIDE = 2048, 9, 8
NCORES = 8
CH = 4  # slots per chunk: 128*CH descriptors/call, <=1024 (SWDGE ring);
        # <=512 keeps two calls in flight so desc-gen overlaps transfers.
FM_ROWS = 4 * 64 * 64  # 4 parity replicas x 64x64 blocks

_PROGRAM_CACHE: dict[int, object] = {}
_PE_PATTERN = lambda s: s % 2 == 0  # which slots take the PE combine path


def _build_program(S: int, iters: int = 1, hw_loop: bool = False):
    import concourse.bacc as bacc
    import concourse.mybir as mybir
    import concourse.tile as tile
    from concourse.bass_types import AP

    f16 = mybir.dt.float16
    f32 = mybir.dt.float32
    i16 = mybir.dt.int16
    i32 = mybir.dt.int32
    op = mybir.AluOpType
    ACT_COPY = mybir.ActivationFunctionType.Copy

    chunks = []
    a = 0
    while a < S:
        b = min(a + CH, S)
        chunks.append((a, b))
        a = b
    if chunks and chunks[-1][1] - chunks[-1][0] >= 4:
        # split the final chunk so the drain tail after the last gather is
        # short (combine+store of 2 slots instead of 4)
        a, b = chunks.pop()
        m = (a + b) // 2
        chunks.extend([(a, m), (m, b)])

    nc = bacc.Bacc("TRN2", target_bir_lowering=False, debug=False, num_devices=NCORES)
    fm_t = nc.dram_tensor("fm", [FM_ROWS, 4 * C], f16, kind="ExternalInput")
    pt_t = nc.dram_tensor("pt", [128, S * 6], f32, kind="ExternalInput")
    id_t = nc.dram_tensor("ident", [128, 128], f16, kind="ExternalInput")
    perm_t = nc.dram_tensor("perm", [128, 8 * 128], f32, kind="ExternalInput")
    # unused chaining token (same shape/dtype as out): lets a timing harness
    # thread out(k) -> tok(k+1) to serialize repeated executions in one jit
    nc.dram_tensor("tok", [128, S * C], f16, kind="ExternalInput")
    out_t = nc.dram_tensor("out", [128, S * C], f16, kind="ExternalOutput")

    fm_gather_ap = AP(fm_t, 0, [[4 * C, FM_ROWS], [1, 4 * C]])

    with tile.TileContext(nc) as tc:
        with (
            tc.tile_pool(name="const", bufs=1) as cpool,
            tc.tile_pool(name="gath", bufs=3) as gpool,
            tc.tile_pool(name="work", bufs=3) as wpool,
            tc.tile_pool(name="outp", bufs=3) as opool,
            tc.tile_pool(name="psum", bufs=4, space="PSUM") as ppool,
        ):
            p128 = cpool.tile([128, S * 6], f32)
            nc.sync.dma_start(p128[:], pt_t[:])
            perm = cpool.tile([128, 8 * 128], f32)
            nc.scalar.dma_start(perm[:], perm_t[:])
            ident = cpool.tile([128, 128], f16)
            nc.scalar.dma_start(ident[:], id_t[:])

            v = p128[:].rearrange("p (q f) -> p q f", f=6)

            def coord_prefix(axis):
                """ixs = sample coord + 16."""
                lo = v[:, :, 0 + axis]
                hi = v[:, :, 2 + axis]
                off = v[:, :, 4 + axis]
                w0 = wpool.tile([128, S], f32, tag=f"w0{axis}")
                nc.vector.tensor_tensor(w0[:], hi, lo, op.subtract)
                sx = wpool.tile([128, S], f32, tag=f"sx{axis}")
                nc.vector.tensor_scalar(sx[:], w0[:], 1.0, 0.1 / STRIDE, op.add, op.mult)
                asum = wpool.tile([128, S], f32, tag=f"as{axis}")
                nc.vector.tensor_tensor(asum[:], lo, hi, op.add)
                ax = wpool.tile([128, S], f32, tag=f"ax{axis}")
                nc.vector.tensor_scalar(ax[:], asum[:], 0.5 / STRIDE, 16.0, op.mult, op.add)
                ixs = wpool.tile([128, S], f32, tag=f"ix{axis}")
                nc.vector.tensor_tensor(ixs[:], off, sx[:], op.mult)
                nc.vector.tensor_tensor(ixs[:], ixs[:], ax[:], op.add)
                return ixs

            def coord_tail(axis, ixs):
                """cif = clip(floor(ixs),16,143); d = edge-masked frac;
                c2 = (cif-16)>>1; par = (cif-16)&1. NOTE: the f32->int
                tensor_copy ROUNDS-to-nearest on HW (CoreSim truncates);
                the is_gt+subtract fixup turns either into a true floor."""
                ci = wpool.tile([128, S], i32, tag=f"ci{axis}")
                nc.vector.tensor_copy(ci[:], ixs[:])
                cif = wpool.tile([128, S], f32, tag=f"cf{axis}")
                nc.vector.tensor_copy(cif[:], ci[:])
                gt = wpool.tile([128, S], f32, tag=f"gt{axis}")
                nc.vector.tensor_tensor(gt[:], cif[:], ixs[:], op.is_gt)
                nc.vector.tensor_tensor(cif[:], cif[:], gt[:], op.subtract)
                nc.vector.tensor_scalar(cif[:], cif[:], 143.0, 16.0, op.min, op.max)
                d = cpool.tile([128, S], f32, tag=f"d{axis}")
                nc.vector.tensor_tensor(d[:], ixs[:], cif[:], op.subtract)
                m = wpool.tile([128, S], f32, tag=f"m{axis}")
                nc.vector.tensor_scalar(m[:], cif[:], 143.0, None, op.is_lt)
                nc.vector.tensor_tensor(d[:], d[:], m[:], op.mult)
                # cl in [0,127]; half + parity (exact small-int f32 math)
                cl = wpool.tile([128, S], f32, tag=f"cl{axis}")
                nc.vector.tensor_scalar(cl[:], cif[:], 16.0, None, op.subtract)
                hf = wpool.tile([128, S], f32, tag=f"hf{axis}")
                nc.vector.tensor_scalar(hf[:], cl[:], 0.5, None, op.mult)
                h2i = wpool.tile([128, S], i32, tag=f"h2i{axis}")
                nc.vector.tensor_copy(h2i[:], hf[:])
                c2 = wpool.tile([128, S], f32, tag=f"c2{axis}")
                nc.vector.tensor_copy(c2[:], h2i[:])
                g2 = wpool.tile([128, S], f32, tag=f"g2{axis}")
                nc.vector.tensor_tensor(g2[:], c2[:], hf[:], op.is_gt)
                nc.vector.tensor_tensor(c2[:], c2[:], g2[:], op.subtract)
                par = wpool.tile([128, S], f32, tag=f"pr{axis}")
                nc.vector.scalar_tensor_tensor(par[:], c2[:], -2.0, cl[:], op.mult, op.add)
                return c2, par, d

            ixs_y = coord_prefix(1)
            ixs_x = coord_prefix(0)
            h2, ph, lh = coord_tail(1, ixs_y)
            w2, pw, lw = coord_tail(0, ixs_x)

            # block row index: (2*ph+pw)*4096 + h2*64 + w2
            pr = wpool.tile([128, S], f32, tag="pr")
            nc.vector.scalar_tensor_tensor(pr[:], ph[:], 2.0, pw[:], op.mult, op.add)
            t0 = wpool.tile([128, S], f32, tag="t0")
            nc.vector.scalar_tensor_tensor(t0[:], h2[:], 64.0, w2[:], op.mult, op.add)
            idxf = wpool.tile([128, S], f32, tag="idxf")
            nc.vector.scalar_tensor_tensor(idxf[:], pr[:], 4096.0, t0[:], op.mult, op.add)

            # Wrap + replicate the indices across the 8 gpsimd stripes with 8
            # one-hot f32 matmuls (exact for ints): psum[:, g, s] =
            # idxf[16g + p%16, s] for every partition p. Avoids the serial
            # DMA-replication latency chain entirely.
            psidx = ppool.tile([128, 8, S], f32, tag="psidx")
            for g in range(8):
                nc.tensor.matmul(
                    psidx[:, g, :], perm[:, g * 128 : (g + 1) * 128], idxf[:],
                    start=True, stop=True)
            # idx16 col layout: 8*s + g (g = p//16), i16
            idx16 = cpool.tile([128, 8 * S], i16)
            idx16v = idx16[:].rearrange("p (s g) -> p s g", g=8)
            for g in range(8):
                nc.vector.tensor_copy(idx16v[:, :, g], psidx[:, g, :])

            # --- bilinear weights; ch for the DVE path, 4 products for PE.
            ch = cpool.tile([128, S], f32)
            nc.vector.tensor_scalar(ch[:], lh[:], -1.0, 1.0, op.mult, op.add)
            w22 = cpool.tile([128, S], f32)
            nc.vector.tensor_tensor(w22[:], lh[:], lw[:], op.mult)
            s1 = wpool.tile([128, S], f32, tag="s1")
            nc.vector.tensor_tensor(s1[:], lh[:], lw[:], op.add)
            w12 = cpool.tile([128, S], f32)
            nc.vector.tensor_tensor(w12[:], lw[:], w22[:], op.subtract)
            w21 = cpool.tile([128, S], f32)
            nc.vector.tensor_tensor(w21[:], lh[:], w22[:], op.subtract)
            w11 = cpool.tile([128, S], f32)
            nc.vector.tensor_tensor(w11[:], w22[:], s1[:], op.subtract)
            nc.vector.tensor_scalar(w11[:], w11[:], 1.0, None, op.add)

            # --- loop-invariant diag(w_k) stacks for the PE-path slots.
            pe_slot = [_PE_PATTERN(s) for s in range(S)]
            wmats = [w11, w12, w21, w22]
            n_pe = sum(pe_slot)
            diags = cpool.tile([128, max(4 * n_pe, 1), 128], f16)
            dslot = {}
            di = 0
            for s in range(S):
                if not pe_slot[s]:
                    continue
                dslot[s] = di
                for k in range(4):
                    # alternate DVE / ACT so diag builds don't starve the
                    # DVE ops that gate the first gather
                    if (4 * di + k) % 2:
                        nc.vector.tensor_scalar(
                            diags[:, 4 * di + k, :], ident[:],
                            wmats[k][:, s : s + 1], None, op.mult,
                        )
                    else:
                        nc.scalar.activation(
                            diags[:, 4 * di + k, :], ident[:], ACT_COPY,
                            bias=0.0, scale=wmats[k][:, s : s + 1],
                        )
                di += 1

            out_v = out_t[:].rearrange("p (s c) -> p s c", c=C)

            def body():
                for (a, b) in chunks:
                    bsz = b - a
                    gt = gpool.tile([128, bsz, 4 * C], f16, tag=f"gt{bsz}")
                    nc.gpsimd.dma_gather(
                        gt[:], fm_gather_ap, idx16[:, 8 * a : 8 * b],
                        128 * bsz, 128 * bsz, 4 * C, elem_step=4 * C,
                    )
                    outc = opool.tile([128, bsz, C], f16, tag=f"oc{bsz}")
                    for s in range(a, b):
                        ds = s - a
                        # tap order in a block: v1=(hl,wl) v2=(hl,wl+1)
                        #                       v3=(hl+1,wl) v4=(hl+1,wl+1)
                        if pe_slot[s]:
                            di = dslot[s]
                            ps = ppool.tile([128, C], f32, tag="ps")
                            for k in range(4):
                                nc.tensor.matmul(
                                    ps[:], diags[:, 4 * di + k, :],
                                    gt[:, ds, k * C : (k + 1) * C],
                                    start=(k == 0), stop=(k == 3))
                            nc.scalar.activation(outc[:, ds, :], ps[:], ACT_COPY)
                        else:
                            top = gt[:, ds, 0 : 2 * C]
                            bot = gt[:, ds, 2 * C : 4 * C]
                            t1 = wpool.tile([128, 2 * C], f16, tag="t1")
                            nc.scalar.activation(
                                t1[:], top, ACT_COPY, bias=0.0,
                                scale=ch[:, s : s + 1],
                            )
                            m2 = wpool.tile([128, 2 * C], f16, tag="m2")
                            nc.vector.tensor_scalar(
                                m2[:], bot, lh[:, s : s + 1], None, op.mult)
                            st = wpool.tile([128, 2 * C], f16, tag="st")
                            nc.vector.tensor_tensor(st[:], t1[:], m2[:], op.add)
                            d = wpool.tile([128, C], f16, tag="dd")
                            nc.vector.tensor_tensor(
                                d[:], st[:, C : 2 * C], st[:, 0:C], op.subtract)
                            e = wpool.tile([128, C], f16, tag="ee")
                            nc.vector.tensor_scalar(
                                e[:], d[:], lw[:, s : s + 1], None, op.mult)
                            nc.vector.tensor_tensor(
                                outc[:, ds, :], e[:], st[:, 0:C], op.add)
                    nc.sync.dma_start(out_v[:, a:b, :], outc[:])

            if hw_loop and iters > 1:
                with tc.For_i(0, iters):
                    body()
            else:
                for _it in range(iters):
                    body()

    nc.compile()
    return nc


def _get_program(S: int):
    if S not in _PROGRAM_CACHE:
        _PROGRAM_CACHE[S] = _build_program(S)
    return _PROGRAM_CACHE[S]


_IDENT = None
_PERM = None


def _perm_mat():
    """R[k, 128*g + m] = 1 iff k == 16*g + (m % 16)."""
    R = np.zeros((128, 8, 128), np.float32)
    for g in range(8):
        for m in range(128):
            R[16 * g + (m % 16), g, m] = 1.0
    return R.reshape(128, 8 * 128)


def _blockify(img_hwc16):
    """[H, W, C] fp16 -> [4*4096, 4*C] parity-replica 2x2-block layout."""
    P = np.zeros((H + 2, W + 2, C), np.float16)
    P[:H, :W] = img_hwc16
    out = np.empty((4, 64, 64, 4 * C), np.float16)
    for ph in (0, 1):
        for pw in (0, 1):
            A = P[ph : ph + 128, pw : pw + 128]
            blk = A.reshape(64, 2, 64, 2, C).transpose(0, 2, 1, 3, 4)
            out[2 * ph + pw] = blk.reshape(64, 64, 4 * C)
    return out.reshape(4 * 4096, 4 * C)


def _host_prep(feat_map, rois, offset, num_point):
    """Route rois by batch index; build per-core inputs."""
    global _IDENT, _PERM
    if _IDENT is None:
        _IDENT = np.eye(128, dtype=np.float16)
        _PERM = _perm_mat()
    bidx = rois[:, 0].astype(np.int32)
    ids = [np.nonzero(bidx == b)[0] for b in range(B)]
    cap = max(len(i) for i in ids)
    S = math.ceil(max(cap * num_point, 1) / 128)

    NP = S * 128
    in_maps = []
    for b in range(B):
        fm_full = _blockify(feat_map[b].transpose(1, 2, 0).astype(np.float16))
        ptdata = np.zeros((NP, 6), np.float32)
        idl = ids[b]
        nb = len(idl)
        if nb:
            r = rois[idl]
            off = offset[idl].reshape(nb, num_point, 2)
            npts = nb * num_point
            ptdata[:npts, 0] = np.repeat(r[:, 1], num_point)
            ptdata[:npts, 1] = np.repeat(r[:, 2], num_point)
            ptdata[:npts, 2] = np.repeat(r[:, 3], num_point)
            ptdata[:npts, 3] = np.repeat(r[:, 4], num_point)
            ptdata[:npts, 4] = off[:, :, 0].reshape(-1)
            ptdata[:npts, 5] = off[:, :, 1].reshape(-1)
        pt128 = np.ascontiguousarray(
            ptdata.reshape(S, 128, 6).transpose(1, 0, 2)
        ).reshape(128, S * 6)
        in_maps.append({"fm": fm_full, "pt": pt128, "ident": _IDENT, "perm": _PERM,
                        "tok": np.zeros((128, S * C), np.float16)})
    return ids, S, in_maps


def _host_unshard(results, ids, S, num_point, n):
    out_full = np.zeros((n, num_point, C), np.float32)
    for b in range(B):
        nb = len(ids[b])
        if not nb:
            continue
        o = results[b]["out"].reshape(128, S, C).transpose(1, 0, 2).reshape(S * 128, C)
        out_full[ids[b]] = o[: nb * num_point].astype(np.float32).reshape(nb, num_point, C)
    return out_full


def kernel(feat_map, rois, offset, stride, num_point, _collect=None):
    from concourse.bass_utils import run_bass_kernel_spmd

    feat_map = np.asarray(feat_map, np.float32)
    rois = np.asarray(rois, np.float32)
    offset = np.asarray(offset, np.float32)
    stride = int(stride)
    num_point = int(num_point)
    assert feat_map.shape == (B, C, H, W), feat_map.shape
    assert stride == STRIDE and num_point == NUM_POINT

    ids, S, in_maps = _host_prep(feat_map, rois, offset, num_point)
    nc = _get_program(S)
    res = run_bass_kernel_spmd(nc, in_maps, core_ids=list(range(NCORES)),
                               **(_collect.pop("spmd_kwargs", {}) if _collect else {}))
    if _collect is not None:
        _collect["res"] = res
    return _host_unshard(res.results, ids, S, num_point, rois.shape[0])


# revision 18
# speedup vs baseline: 1.1429x; 1.1429x over previous
"""Deformable-ROI bilinear feature gather (KeypPointBboxNet) on 8 TRN2 cores.

v3: feat_map sharded on batch (one image per core), stored as FOUR
parity-shifted 2x2-pixel-block layouts in fp16: replica (ph,pw), block
(h2,w2) holds pixels (2*h2+ph+{0,1}, 2*w2+pw+{0,1}) x 256ch = 2KB
contiguous. A bilinear sample's 4 taps always live in exactly one block
of the replica matching (hl&1, wl&1), so each point is ONE 2KB gather
descriptor (full DMA-bus rate; half the SWDGE descriptor load of a
row-pair scheme).

Per core:
  - coordinate/weight pipeline in [128, S] f32 on DVE,
  - block index = (2*ph+pw)*4096 + (hl>>1)*64 + (wl>>1), shuffled to the
    16-partition-wrapped gather layout via 8 small DMAs + 3 replications,
  - combine alternates PE path (4 accumulating diag(w_k) @ v_k matmuls
    into PSUM, diags prebuilt once on DVE, ACT evacuates) with a DVE/ACT
    fp16 path, per slot,
  - per-chunk fp16 stores overlap the next chunk's gather.
"""

import math

import numpy as np

B, C, H, W = 8, 256, 128, 128
N_ROIS, NUM_POINT, STRIDE = 2048, 9, 8
NCORES = 8
CH = 4  # slots per chunk: 128*CH descriptors/call, <=1024 (SWDGE ring);
        # <=512 keeps two calls in flight so desc-gen overlaps transfers.
FM_ROWS = 4 * 64 * 64  # 4 parity replicas x 64x64 blocks

_PROGRAM_CACHE: dict[int, object] = {}
_PE_PATTERN = lambda s: s % 2 == 0  # which slots take the PE combine path


def _build_program(S: int, iters: int = 1, hw_loop: bool = False):
    import concourse.bacc as bacc
    import concourse.mybir as mybir
    import concourse.tile as tile
    from concourse.bass_types import AP

    f16 = mybir.dt.float16
    f32 = mybir.dt.float32
    i16 = mybir.dt.int16
    i32 = mybir.dt.int32
    op = mybir.AluOpType
    ACT_COPY = mybir.ActivationFunctionType.Copy

    chunks = []
    a = 0
    while a < S:
        b = min(a + CH, S)
        chunks.append((a, b))
        a = b
    if chunks and chunks[-1][1] - chunks[-1][0] >= 4:
        # split the final chunk so the drain tail after the last gather is
        # short (combine+store of 2 slots instead of 4)
        a, b = chunks.pop()
        m = (a + b) // 2
        chunks.extend([(a, m), (m, b)])

    nc = bacc.Bacc("TRN2", target_bir_lowering=False, debug=False, num_devices=NCORES)
    fm_t = nc.dram_tensor("fm", [FM_ROWS, 4 * C], f16, kind="ExternalInput")
    pt_t = nc.dram_tensor("pt", [128, S * 6], f32, kind="ExternalInput")
    id_t = nc.dram_tensor("ident", [128, 128], f16, kind="ExternalInput")
    perm_t = nc.dram_tensor("perm", [128, 8 * 128], f32, kind="ExternalInput")
    # unused chaining token (same shape/dtype as out): lets a timing harness
    # thread out(k) -> tok(k+1) to serialize repeated executions in one jit
    nc.dram_tensor("tok", [128, S * C], f16, kind="ExternalInput")
    out_t = nc.dram_tensor("out", [128, S * C], f16, kind="ExternalOutput")

    fm_gather_ap = AP(fm_t, 0, [[4 * C, FM_ROWS], [1, 4 * C]])

    with tile.TileContext(nc) as tc:
        with (
            tc.tile_pool(name="const", bufs=1) as cpool,
            tc.tile_pool(name="gath", bufs=4) as gpool,
            tc.tile_pool(name="work", bufs=4) as wpool,
            tc.tile_pool(name="outp", bufs=4) as opool,
            tc.tile_pool(name="psum", bufs=4, space="PSUM") as ppool,
        ):
            p128 = cpool.tile([128, S * 6], f32)
            nc.sync.dma_start(p128[:], pt_t[:])
            perm = cpool.tile([128, 8 * 128], f32)
            nc.scalar.dma_start(perm[:], perm_t[:])
            ident = cpool.tile([128, 128], f16)
            nc.scalar.dma_start(ident[:], id_t[:])

            v = p128[:].rearrange("p (q f) -> p q f", f=6)

            def coord_prefix(axis):
                """ixs = sample coord + 16."""
                lo = v[:, :, 0 + axis]
                hi = v[:, :, 2 + axis]
                off = v[:, :, 4 + axis]
                w0 = wpool.tile([128, S], f32, tag=f"w0{axis}")
                nc.vector.tensor_tensor(w0[:], hi, lo, op.subtract)
                sx = wpool.tile([128, S], f32, tag=f"sx{axis}")
                nc.vector.tensor_scalar(sx[:], w0[:], 1.0, 0.1 / STRIDE, op.add, op.mult)
                asum = wpool.tile([128, S], f32, tag=f"as{axis}")
                nc.vector.tensor_tensor(asum[:], lo, hi, op.add)
                ax = wpool.tile([128, S], f32, tag=f"ax{axis}")
                nc.vector.tensor_scalar(ax[:], asum[:], 0.5 / STRIDE, 16.0, op.mult, op.add)
                ixs = wpool.tile([128, S], f32, tag=f"ix{axis}")
                nc.vector.tensor_tensor(ixs[:], off, sx[:], op.mult)
                nc.vector.tensor_tensor(ixs[:], ixs[:], ax[:], op.add)
                return ixs

            def coord_tail(axis, ixs):
                """cif = clip(floor(ixs),16,143); d = edge-masked frac;
                c2 = (cif-16)>>1; par = (cif-16)&1. NOTE: the f32->int
                tensor_copy ROUNDS-to-nearest on HW (CoreSim truncates);
                the is_gt+subtract fixup turns either into a true floor."""
                ci = wpool.tile([128, S], i32, tag=f"ci{axis}")
                nc.vector.tensor_copy(ci[:], ixs[:])
                cif = wpool.tile([128, S], f32, tag=f"cf{axis}")
                nc.vector.tensor_copy(cif[:], ci[:])
                gt = wpool.tile([128, S], f32, tag=f"gt{axis}")
                nc.vector.tensor_tensor(gt[:], cif[:], ixs[:], op.is_gt)
                nc.vector.tensor_tensor(cif[:], cif[:], gt[:], op.subtract)
                nc.vector.tensor_scalar(cif[:], cif[:], 143.0, 16.0, op.min, op.max)
                d = cpool.tile([128, S], f32, tag=f"d{axis}")
                nc.vector.tensor_tensor(d[:], ixs[:], cif[:], op.subtract)
                m = wpool.tile([128, S], f32, tag=f"m{axis}")
                nc.vector.tensor_scalar(m[:], cif[:], 143.0, None, op.is_lt)
                nc.vector.tensor_tensor(d[:], d[:], m[:], op.mult)
                # cl in [0,127]; half + parity (exact small-int f32 math)
                cl = wpool.tile([128, S], f32, tag=f"cl{axis}")
                nc.vector.tensor_scalar(cl[:], cif[:], 16.0, None, op.subtract)
                hf = wpool.tile([128, S], f32, tag=f"hf{axis}")
                nc.vector.tensor_scalar(hf[:], cl[:], 0.5, None, op.mult)
                h2i = wpool.tile([128, S], i32, tag=f"h2i{axis}")
                nc.vector.tensor_copy(h2i[:], hf[:])
                c2 = wpool.tile([128, S], f32, tag=f"c2{axis}")
                nc.vector.tensor_copy(c2[:], h2i[:])
                g2 = wpool.tile([128, S], f32, tag=f"g2{axis}")
                nc.vector.tensor_tensor(g2[:], c2[:], hf[:], op.is_gt)
                nc.vector.tensor_tensor(c2[:], c2[:], g2[:], op.subtract)
                par = wpool.tile([128, S], f32, tag=f"pr{axis}")
                nc.vector.scalar_tensor_tensor(par[:], c2[:], -2.0, cl[:], op.mult, op.add)
                return c2, par, d

            ixs_y = coord_prefix(1)
            ixs_x = coord_prefix(0)
            h2, ph, lh = coord_tail(1, ixs_y)
            w2, pw, lw = coord_tail(0, ixs_x)

            # block row index: (2*ph+pw)*4096 + h2*64 + w2
            pr = wpool.tile([128, S], f32, tag="pr")
            nc.vector.scalar_tensor_tensor(pr[:], ph[:], 2.0, pw[:], op.mult, op.add)
            t0 = wpool.tile([128, S], f32, tag="t0")
            nc.vector.scalar_tensor_tensor(t0[:], h2[:], 64.0, w2[:], op.mult, op.add)
            idxf = wpool.tile([128, S], f32, tag="idxf")
            nc.vector.scalar_tensor_tensor(idxf[:], pr[:], 4096.0, t0[:], op.mult, op.add)

            # Wrap + replicate the indices across the 8 gpsimd stripes with 8
            # one-hot f32 matmuls (exact for ints): psum[:, g, s] =
            # idxf[16g + p%16, s] for every partition p. Avoids the serial
            # DMA-replication latency chain entirely.
            psidx = ppool.tile([128, 8, S], f32, tag="psidx")
            for g in range(8):
                nc.tensor.matmul(
                    psidx[:, g, :], perm[:, g * 128 : (g + 1) * 128], idxf[:],
                    start=True, stop=True)
            # idx16 col layout: 8*s + g (g = p//16), i16
            idx16 = cpool.tile([128, 8 * S], i16)
            idx16v = idx16[:].rearrange("p (s g) -> p s g", g=8)
            for g in range(8):
                nc.vector.tensor_copy(idx16v[:, :, g], psidx[:, g, :])

            # --- bilinear weights; ch for the DVE path, 4 products for PE.
            ch = cpool.tile([128, S], f32)
            nc.vector.tensor_scalar(ch[:], lh[:], -1.0, 1.0, op.mult, op.add)
            w22 = cpool.tile([128, S], f32)
            nc.vector.tensor_tensor(w22[:], lh[:], lw[:], op.mult)
            s1 = wpool.tile([128, S], f32, tag="s1")
            nc.vector.tensor_tensor(s1[:], lh[:], lw[:], op.add)
            w12 = cpool.tile([128, S], f32)
            nc.vector.tensor_tensor(w12[:], lw[:], w22[:], op.subtract)
            w21 = cpool.tile([128, S], f32)
            nc.vector.tensor_tensor(w21[:], lh[:], w22[:], op.subtract)
            w11 = cpool.tile([128, S], f32)
            nc.vector.tensor_tensor(w11[:], w22[:], s1[:], op.subtract)
            nc.vector.tensor_scalar(w11[:], w11[:], 1.0, None, op.add)

            # --- loop-invariant diag(w_k) stacks for the PE-path slots.
            pe_slot = [_PE_PATTERN(s) for s in range(S)]
            wmats = [w11, w12, w21, w22]
            n_pe = sum(pe_slot)
            diags = cpool.tile([128, max(4 * n_pe, 1), 128], f16)
            dslot = {}
            di = 0
            for s in range(S):
                if not pe_slot[s]:
                    continue
                dslot[s] = di
                for k in range(4):
                    # alternate DVE / ACT so diag builds don't starve the
                    # DVE ops that gate the first gather
                    if (4 * di + k) % 2:
                        nc.vector.tensor_scalar(
                            diags[:, 4 * di + k, :], ident[:],
                            wmats[k][:, s : s + 1], None, op.mult,
                        )
                    else:
                        nc.scalar.activation(
                            diags[:, 4 * di + k, :], ident[:], ACT_COPY,
                            bias=0.0, scale=wmats[k][:, s : s + 1],
                        )
                di += 1

            out_v = out_t[:].rearrange("p (s c) -> p s c", c=C)

            def body():
                for (a, b) in chunks:
                    bsz = b - a
                    gt = gpool.tile([128, bsz, 4 * C], f16, tag=f"gt{bsz}")
                    nc.gpsimd.dma_gather(
                        gt[:], fm_gather_ap, idx16[:, 8 * a : 8 * b],
                        128 * bsz, 128 * bsz, 4 * C, elem_step=4 * C,
                    )
                    outc = opool.tile([128, bsz, C], f16, tag=f"oc{bsz}")
                    for s in range(a, b):
                        ds = s - a
                        # tap order in a block: v1=(hl,wl) v2=(hl,wl+1)
                        #                       v3=(hl+1,wl) v4=(hl+1,wl+1)
                        if pe_slot[s]:
                            di = dslot[s]
                            ps = ppool.tile([128, C], f32, tag="ps")
                            for k in range(4):
                                nc.tensor.matmul(
                                    ps[:], diags[:, 4 * di + k, :],
                                    gt[:, ds, k * C : (k + 1) * C],
                                    start=(k == 0), stop=(k == 3))
                            nc.scalar.activation(outc[:, ds, :], ps[:], ACT_COPY)
                        else:
                            top = gt[:, ds, 0 : 2 * C]
                            bot = gt[:, ds, 2 * C : 4 * C]
                            t1 = wpool.tile([128, 2 * C], f16, tag="t1")
                            nc.scalar.activation(
                                t1[:], top, ACT_COPY, bias=0.0,
                                scale=ch[:, s : s + 1],
                            )
                            m2 = wpool.tile([128, 2 * C], f16, tag="m2")
                            nc.vector.tensor_scalar(
                                m2[:], bot, lh[:, s : s + 1], None, op.mult)
                            st = wpool.tile([128, 2 * C], f16, tag="st")
                            nc.vector.tensor_tensor(st[:], t1[:], m2[:], op.add)
                            d = wpool.tile([128, C], f16, tag="dd")
                            nc.vector.tensor_tensor(
                                d[:], st[:, C : 2 * C], st[:, 0:C], op.subtract)
                            e = wpool.tile([128, C], f16, tag="ee")
                            nc.vector.tensor_scalar(
                                e[:], d[:], lw[:, s : s + 1], None, op.mult)
                            nc.vector.tensor_tensor(
                                outc[:, ds, :], e[:], st[:, 0:C], op.add)
                    nc.sync.dma_start(out_v[:, a:b, :], outc[:])

            if hw_loop and iters > 1:
                # unroll several bodies per loop iteration so the back-edge
                # pipeline drain amortizes and the marginal cost approaches
                # the overlapped steady-state body cost
                UNROLL = 8
                assert iters % UNROLL == 0
                with tc.For_i(0, iters // UNROLL):
                    for _ in range(UNROLL):
                        body()
            else:
                for _it in range(iters):
                    body()

    nc.compile()
    return nc


def _get_program(S: int):
    if S not in _PROGRAM_CACHE:
        _PROGRAM_CACHE[S] = _build_program(S)
    return _PROGRAM_CACHE[S]


_IDENT = None
_PERM = None


def _perm_mat():
    """R[k, 128*g + m] = 1 iff k == 16*g + (m % 16)."""
    R = np.zeros((128, 8, 128), np.float32)
    for g in range(8):
        for m in range(128):
            R[16 * g + (m % 16), g, m] = 1.0
    return R.reshape(128, 8 * 128)


def _blockify(img_hwc16):
    """[H, W, C] fp16 -> [4*4096, 4*C] parity-replica 2x2-block layout."""
    P = np.zeros((H + 2, W + 2, C), np.float16)
    P[:H, :W] = img_hwc16
    out = np.empty((4, 64, 64, 4 * C), np.float16)
    for ph in (0, 1):
        for pw in (0, 1):
            A = P[ph : ph + 128, pw : pw + 128]
            blk = A.reshape(64, 2, 64, 2, C).transpose(0, 2, 1, 3, 4)
            out[2 * ph + pw] = blk.reshape(64, 64, 4 * C)
    return out.reshape(4 * 4096, 4 * C)


def _host_prep(feat_map, rois, offset, num_point):
    """Route rois by batch index; build per-core inputs."""
    global _IDENT, _PERM
    if _IDENT is None:
        _IDENT = np.eye(128, dtype=np.float16)
        _PERM = _perm_mat()
    bidx = rois[:, 0].astype(np.int32)
    ids = [np.nonzero(bidx == b)[0] for b in range(B)]
    cap = max(len(i) for i in ids)
    S = math.ceil(max(cap * num_point, 1) / 128)

    NP = S * 128
    in_maps = []
    for b in range(B):
        fm_full = _blockify(feat_map[b].transpose(1, 2, 0).astype(np.float16))
        ptdata = np.zeros((NP, 6), np.float32)
        idl = ids[b]
        nb = len(idl)
        if nb:
            r = rois[idl]
            off = offset[idl].reshape(nb, num_point, 2)
            npts = nb * num_point
            ptdata[:npts, 0] = np.repeat(r[:, 1], num_point)
            ptdata[:npts, 1] = np.repeat(r[:, 2], num_point)
            ptdata[:npts, 2] = np.repeat(r[:, 3], num_point)
            ptdata[:npts, 3] = np.repeat(r[:, 4], num_point)
            ptdata[:npts, 4] = off[:, :, 0].reshape(-1)
            ptdata[:npts, 5] = off[:, :, 1].reshape(-1)
        pt128 = np.ascontiguousarray(
            ptdata.reshape(S, 128, 6).transpose(1, 0, 2)
        ).reshape(128, S * 6)
        in_maps.append({"fm": fm_full, "pt": pt128, "ident": _IDENT, "perm": _PERM,
                        "tok": np.zeros((128, S * C), np.float16)})
    return ids, S, in_maps


def _host_unshard(results, ids, S, num_point, n):
    out_full = np.zeros((n, num_point, C), np.float32)
    for b in range(B):
        nb = len(ids[b])
        if not nb:
            continue
        o = results[b]["out"].reshape(128, S, C).transpose(1, 0, 2).reshape(S * 128, C)
        out_full[ids[b]] = o[: nb * num_point].astype(np.float32).reshape(nb, num_point, C)
    return out_full


def kernel(feat_map, rois, offset, stride, num_point, _collect=None):
    from concourse.bass_utils import run_bass_kernel_spmd

    feat_map = np.asarray(feat_map, np.float32)
    rois = np.asarray(rois, np.float32)
    offset = np.asarray(offset, np.float32)
    stride = int(stride)
    num_point = int(num_point)
    assert feat_map.shape == (B, C, H, W), feat_map.shape
    assert stride == STRIDE and num_point == NUM_POINT

    ids, S, in_maps = _host_prep(feat_map, rois, offset, num_point)
    nc = _get_program(S)
    res = run_bass_kernel_spmd(nc, in_maps, core_ids=list(range(NCORES)),
                               **(_collect.pop("spmd_kwargs", {}) if _collect else {}))
    if _collect is not None:
        _collect["res"] = res
    return _host_unshard(res.results, ids, S, num_point, rois.shape[0])


# revision 20
# speedup vs baseline: 1.2211x; 1.0684x over previous
"""Deformable-ROI bilinear feature gather (KeypPointBboxNet) on 8 TRN2 cores.

v3: feat_map sharded on batch (one image per core), stored as FOUR
parity-shifted 2x2-pixel-block layouts in fp16: replica (ph,pw), block
(h2,w2) holds pixels (2*h2+ph+{0,1}, 2*w2+pw+{0,1}) x 256ch = 2KB
contiguous. A bilinear sample's 4 taps always live in exactly one block
of the replica matching (hl&1, wl&1), so each point is ONE 2KB gather
descriptor (full DMA-bus rate; half the SWDGE descriptor load of a
row-pair scheme).

Per core:
  - coordinate/weight pipeline in [128, S] f32 on DVE,
  - block index = (2*ph+pw)*4096 + (hl>>1)*64 + (wl>>1), shuffled to the
    16-partition-wrapped gather layout via 8 small DMAs + 3 replications,
  - combine alternates PE path (4 accumulating diag(w_k) @ v_k matmuls
    into PSUM, diags prebuilt once on DVE, ACT evacuates) with a DVE/ACT
    fp16 path, per slot,
  - per-chunk fp16 stores overlap the next chunk's gather.
"""

import math

import numpy as np

B, C, H, W = 8, 256, 128, 128
N_ROIS, NUM_POINT, STRIDE = 2048, 9, 8
NCORES = 8
CH = 4  # slots per chunk: 128*CH descriptors/call, <=1024 (SWDGE ring);
        # <=512 keeps two calls in flight so desc-gen overlaps transfers.
FM_ROWS = 4 * 64 * 64  # 4 parity replicas x 64x64 blocks

_PROGRAM_CACHE: dict[int, object] = {}
_PE_PATTERN = lambda s: s % 2 == 0  # which slots take the PE combine path


def _build_program(S: int, iters: int = 1, hw_loop: bool = False):
    import concourse.bacc as bacc
    import concourse.mybir as mybir
    import concourse.tile as tile
    from concourse.bass_types import AP

    f16 = mybir.dt.float16
    f32 = mybir.dt.float32
    i16 = mybir.dt.int16
    i32 = mybir.dt.int32
    op = mybir.AluOpType
    ACT_COPY = mybir.ActivationFunctionType.Copy

    chunks = []
    a = 0
    while a < S:
        b = min(a + CH, S)
        chunks.append((a, b))
        a = b
    if chunks and chunks[-1][1] - chunks[-1][0] >= 4:
        # split the final chunk so the drain tail after the last gather is
        # short (combine+store of 2 slots instead of 4)
        a, b = chunks.pop()
        m = (a + b) // 2
        chunks.extend([(a, m), (m, b)])

    nc = bacc.Bacc("TRN2", target_bir_lowering=False, debug=False, num_devices=NCORES,
                   num_swdge_queues=2)
    fm_t = nc.dram_tensor("fm", [FM_ROWS, 4 * C], f16, kind="ExternalInput")
    pt_t = nc.dram_tensor("pt", [128, S * 6], f32, kind="ExternalInput")
    id_t = nc.dram_tensor("ident", [128, 128], f16, kind="ExternalInput")
    perm_t = nc.dram_tensor("perm", [128, 8 * 128], f32, kind="ExternalInput")
    # unused chaining token (same shape/dtype as out): lets a timing harness
    # thread out(k) -> tok(k+1) to serialize repeated executions in one jit
    nc.dram_tensor("tok", [128, S * C], f16, kind="ExternalInput")
    out_t = nc.dram_tensor("out", [128, S * C], f16, kind="ExternalOutput")

    fm_gather_ap = AP(fm_t, 0, [[4 * C, FM_ROWS], [1, 4 * C]])

    with tile.TileContext(nc) as tc:
        with (
            tc.tile_pool(name="const", bufs=1) as cpool,
            tc.tile_pool(name="gath", bufs=4) as gpool,
            tc.tile_pool(name="work", bufs=4) as wpool,
            tc.tile_pool(name="outp", bufs=4) as opool,
            tc.tile_pool(name="psum", bufs=4, space="PSUM") as ppool,
        ):
            p128 = cpool.tile([128, S * 6], f32)
            nc.sync.dma_start(p128[:], pt_t[:])
            perm = cpool.tile([128, 8 * 128], f32)
            nc.scalar.dma_start(perm[:], perm_t[:])
            ident = cpool.tile([128, 128], f16)
            nc.scalar.dma_start(ident[:], id_t[:])

            v = p128[:].rearrange("p (q f) -> p q f", f=6)

            def coord_prefix(axis):
                """ixs = sample coord + 16."""
                lo = v[:, :, 0 + axis]
                hi = v[:, :, 2 + axis]
                off = v[:, :, 4 + axis]
                w0 = wpool.tile([128, S], f32, tag=f"w0{axis}")
                nc.vector.tensor_tensor(w0[:], hi, lo, op.subtract)
                sx = wpool.tile([128, S], f32, tag=f"sx{axis}")
                nc.vector.tensor_scalar(sx[:], w0[:], 1.0, 0.1 / STRIDE, op.add, op.mult)
                asum = wpool.tile([128, S], f32, tag=f"as{axis}")
                nc.vector.tensor_tensor(asum[:], lo, hi, op.add)
                ax = wpool.tile([128, S], f32, tag=f"ax{axis}")
                nc.vector.tensor_scalar(ax[:], asum[:], 0.5 / STRIDE, 16.0, op.mult, op.add)
                ixs = wpool.tile([128, S], f32, tag=f"ix{axis}")
                nc.vector.tensor_tensor(ixs[:], off, sx[:], op.mult)
                nc.vector.tensor_tensor(ixs[:], ixs[:], ax[:], op.add)
                return ixs

            def coord_tail(axis, ixs):
                """cif = clip(floor(ixs),16,143); d = edge-masked frac;
                c2 = (cif-16)>>1; par = (cif-16)&1. NOTE: the f32->int
                tensor_copy ROUNDS-to-nearest on HW (CoreSim truncates);
                the is_gt+subtract fixup turns either into a true floor."""
                ci = wpool.tile([128, S], i32, tag=f"ci{axis}")
                nc.vector.tensor_copy(ci[:], ixs[:])
                cif = wpool.tile([128, S], f32, tag=f"cf{axis}")
                nc.vector.tensor_copy(cif[:], ci[:])
                gt = wpool.tile([128, S], f32, tag=f"gt{axis}")
                nc.vector.tensor_tensor(gt[:], cif[:], ixs[:], op.is_gt)
                nc.vector.tensor_tensor(cif[:], cif[:], gt[:], op.subtract)
                nc.vector.tensor_scalar(cif[:], cif[:], 143.0, 16.0, op.min, op.max)
                d = cpool.tile([128, S], f32, tag=f"d{axis}")
                nc.vector.tensor_tensor(d[:], ixs[:], cif[:], op.subtract)
                m = wpool.tile([128, S], f32, tag=f"m{axis}")
                nc.vector.tensor_scalar(m[:], cif[:], 143.0, None, op.is_lt)
                nc.vector.tensor_tensor(d[:], d[:], m[:], op.mult)
                # cl in [0,127]; half + parity (exact small-int f32 math)
                cl = wpool.tile([128, S], f32, tag=f"cl{axis}")
                nc.vector.tensor_scalar(cl[:], cif[:], 16.0, None, op.subtract)
                hf = wpool.tile([128, S], f32, tag=f"hf{axis}")
                nc.vector.tensor_scalar(hf[:], cl[:], 0.5, None, op.mult)
                h2i = wpool.tile([128, S], i32, tag=f"h2i{axis}")
                nc.vector.tensor_copy(h2i[:], hf[:])
                c2 = wpool.tile([128, S], f32, tag=f"c2{axis}")
                nc.vector.tensor_copy(c2[:], h2i[:])
                g2 = wpool.tile([128, S], f32, tag=f"g2{axis}")
                nc.vector.tensor_tensor(g2[:], c2[:], hf[:], op.is_gt)
                nc.vector.tensor_tensor(c2[:], c2[:], g2[:], op.subtract)
                par = wpool.tile([128, S], f32, tag=f"pr{axis}")
                nc.vector.scalar_tensor_tensor(par[:], c2[:], -2.0, cl[:], op.mult, op.add)
                return c2, par, d

            ixs_y = coord_prefix(1)
            ixs_x = coord_prefix(0)
            h2, ph, lh = coord_tail(1, ixs_y)
            w2, pw, lw = coord_tail(0, ixs_x)

            # block row index: (2*ph+pw)*4096 + h2*64 + w2
            pr = wpool.tile([128, S], f32, tag="pr")
            nc.vector.scalar_tensor_tensor(pr[:], ph[:], 2.0, pw[:], op.mult, op.add)
            t0 = wpool.tile([128, S], f32, tag="t0")
            nc.vector.scalar_tensor_tensor(t0[:], h2[:], 64.0, w2[:], op.mult, op.add)
            idxf = wpool.tile([128, S], f32, tag="idxf")
            nc.vector.scalar_tensor_tensor(idxf[:], pr[:], 4096.0, t0[:], op.mult, op.add)

            # Wrap + replicate the indices across the 8 gpsimd stripes with 8
            # one-hot f32 matmuls (exact for ints): psum[:, g, s] =
            # idxf[16g + p%16, s] for every partition p. Avoids the serial
            # DMA-replication latency chain entirely.
            psidx = ppool.tile([128, 8, S], f32, tag="psidx")
            for g in range(8):
                nc.tensor.matmul(
                    psidx[:, g, :], perm[:, g * 128 : (g + 1) * 128], idxf[:],
                    start=True, stop=True)
            # idx16 col layout: 8*s + g (g = p//16), i16
            idx16 = cpool.tile([128, 8 * S], i16)
            idx16v = idx16[:].rearrange("p (s g) -> p s g", g=8)
            for g in range(8):
                nc.vector.tensor_copy(idx16v[:, :, g], psidx[:, g, :])

            # --- bilinear weights; ch for the DVE path, 4 products for PE.
            ch = cpool.tile([128, S], f32)
            nc.vector.tensor_scalar(ch[:], lh[:], -1.0, 1.0, op.mult, op.add)
            w22 = cpool.tile([128, S], f32)
            nc.vector.tensor_tensor(w22[:], lh[:], lw[:], op.mult)
            s1 = wpool.tile([128, S], f32, tag="s1")
            nc.vector.tensor_tensor(s1[:], lh[:], lw[:], op.add)
            w12 = cpool.tile([128, S], f32)
            nc.vector.tensor_tensor(w12[:], lw[:], w22[:], op.subtract)
            w21 = cpool.tile([128, S], f32)
            nc.vector.tensor_tensor(w21[:], lh[:], w22[:], op.subtract)
            w11 = cpool.tile([128, S], f32)
            nc.vector.tensor_tensor(w11[:], w22[:], s1[:], op.subtract)
            nc.vector.tensor_scalar(w11[:], w11[:], 1.0, None, op.add)

            # --- loop-invariant diag(w_k) stacks for the PE-path slots.
            pe_slot = [_PE_PATTERN(s) for s in range(S)]
            wmats = [w11, w12, w21, w22]
            n_pe = sum(pe_slot)
            diags = cpool.tile([128, max(4 * n_pe, 1), 128], f16)
            dslot = {}
            di = 0
            for s in range(S):
                if not pe_slot[s]:
                    continue
                dslot[s] = di
                for k in range(4):
                    # alternate DVE / ACT so diag builds don't starve the
                    # DVE ops that gate the first gather
                    if (4 * di + k) % 2:
                        nc.vector.tensor_scalar(
                            diags[:, 4 * di + k, :], ident[:],
                            wmats[k][:, s : s + 1], None, op.mult,
                        )
                    else:
                        nc.scalar.activation(
                            diags[:, 4 * di + k, :], ident[:], ACT_COPY,
                            bias=0.0, scale=wmats[k][:, s : s + 1],
                        )
                di += 1

            out_v = out_t[:].rearrange("p (s c) -> p s c", c=C)

            def body():
                for ci_, (a, b) in enumerate(chunks):
                    bsz = b - a
                    gt = gpool.tile([128, bsz, 4 * C], f16, tag=f"gt{bsz}")
                    nc.gpsimd.dma_gather(
                        gt[:], fm_gather_ap, idx16[:, 8 * a : 8 * b],
                        128 * bsz, 128 * bsz, 4 * C, elem_step=4 * C,
                        queue_num=ci_ % 2,
                    )
                    outc = opool.tile([128, bsz, C], f16, tag=f"oc{bsz}")
                    for s in range(a, b):
                        ds = s - a
                        # tap order in a block: v1=(hl,wl) v2=(hl,wl+1)
                        #                       v3=(hl+1,wl) v4=(hl+1,wl+1)
                        if pe_slot[s]:
                            di = dslot[s]
                            ps = ppool.tile([128, C], f32, tag="ps")
                            for k in range(4):
                                nc.tensor.matmul(
                                    ps[:], diags[:, 4 * di + k, :],
                                    gt[:, ds, k * C : (k + 1) * C],
                                    start=(k == 0), stop=(k == 3))
                            nc.scalar.activation(outc[:, ds, :], ps[:], ACT_COPY)
                        else:
                            top = gt[:, ds, 0 : 2 * C]
                            bot = gt[:, ds, 2 * C : 4 * C]
                            t1 = wpool.tile([128, 2 * C], f16, tag="t1")
                            nc.scalar.activation(
                                t1[:], top, ACT_COPY, bias=0.0,
                                scale=ch[:, s : s + 1],
                            )
                            m2 = wpool.tile([128, 2 * C], f16, tag="m2")
                            nc.vector.tensor_scalar(
                                m2[:], bot, lh[:, s : s + 1], None, op.mult)
                            st = wpool.tile([128, 2 * C], f16, tag="st")
                            nc.vector.tensor_tensor(st[:], t1[:], m2[:], op.add)
                            d = wpool.tile([128, C], f16, tag="dd")
                            nc.vector.tensor_tensor(
                                d[:], st[:, C : 2 * C], st[:, 0:C], op.subtract)
                            e = wpool.tile([128, C], f16, tag="ee")
                            nc.vector.tensor_scalar(
                                e[:], d[:], lw[:, s : s + 1], None, op.mult)
                            nc.vector.tensor_tensor(
                                outc[:, ds, :], e[:], st[:, 0:C], op.add)
                    nc.sync.dma_start(out_v[:, a:b, :], outc[:])

            if hw_loop and iters > 1:
                # unroll several bodies per loop iteration so the back-edge
                # pipeline drain amortizes and the marginal cost approaches
                # the overlapped steady-state body cost
                UNROLL = 8
                assert iters % UNROLL == 0
                with tc.For_i(0, iters // UNROLL):
                    for _ in range(UNROLL):
                        body()
            else:
                for _it in range(iters):
                    body()

    nc.compile()
    return nc


def _get_program(S: int):
    if S not in _PROGRAM_CACHE:
        _PROGRAM_CACHE[S] = _build_program(S)
    return _PROGRAM_CACHE[S]


_IDENT = None
_PERM = None


def _perm_mat():
    """R[k, 128*g + m] = 1 iff k == 16*g + (m % 16)."""
    R = np.zeros((128, 8, 128), np.float32)
    for g in range(8):
        for m in range(128):
            R[16 * g + (m % 16), g, m] = 1.0
    return R.reshape(128, 8 * 128)


def _blockify(img_hwc16):
    """[H, W, C] fp16 -> [4*4096, 4*C] parity-replica 2x2-block layout."""
    P = np.zeros((H + 2, W + 2, C), np.float16)
    P[:H, :W] = img_hwc16
    out = np.empty((4, 64, 64, 4 * C), np.float16)
    for ph in (0, 1):
        for pw in (0, 1):
            A = P[ph : ph + 128, pw : pw + 128]
            blk = A.reshape(64, 2, 64, 2, C).transpose(0, 2, 1, 3, 4)
            out[2 * ph + pw] = blk.reshape(64, 64, 4 * C)
    return out.reshape(4 * 4096, 4 * C)


def _host_prep(feat_map, rois, offset, num_point):
    """Route rois by batch index; build per-core inputs."""
    global _IDENT, _PERM
    if _IDENT is None:
        _IDENT = np.eye(128, dtype=np.float16)
        _PERM = _perm_mat()
    bidx = rois[:, 0].astype(np.int32)
    ids = [np.nonzero(bidx == b)[0] for b in range(B)]
    cap = max(len(i) for i in ids)
    S = math.ceil(max(cap * num_point, 1) / 128)

    NP = S * 128
    in_maps = []
    for b in range(B):
        fm_full = _blockify(feat_map[b].transpose(1, 2, 0).astype(np.float16))
        ptdata = np.zeros((NP, 6), np.float32)
        idl = ids[b]
        nb = len(idl)
        if nb:
            r = rois[idl]
            off = offset[idl].reshape(nb, num_point, 2)
            npts = nb * num_point
            ptdata[:npts, 0] = np.repeat(r[:, 1], num_point)
            ptdata[:npts, 1] = np.repeat(r[:, 2], num_point)
            ptdata[:npts, 2] = np.repeat(r[:, 3], num_point)
            ptdata[:npts, 3] = np.repeat(r[:, 4], num_point)
            ptdata[:npts, 4] = off[:, :, 0].reshape(-1)
            ptdata[:npts, 5] = off[:, :, 1].reshape(-1)
        pt128 = np.ascontiguousarray(
            ptdata.reshape(S, 128, 6).transpose(1, 0, 2)
        ).reshape(128, S * 6)
        in_maps.append({"fm": fm_full, "pt": pt128, "ident": _IDENT, "perm": _PERM,
                        "tok": np.zeros((128, S * C), np.float16)})
    return ids, S, in_maps


def _host_unshard(results, ids, S, num_point, n):
    out_full = np.zeros((n, num_point, C), np.float32)
    for b in range(B):
        nb = len(ids[b])
        if not nb:
            continue
        o = results[b]["out"].reshape(128, S, C).transpose(1, 0, 2).reshape(S * 128, C)
        out_full[ids[b]] = o[: nb * num_point].astype(np.float32).reshape(nb, num_point, C)
    return out_full


def kernel(feat_map, rois, offset, stride, num_point, _collect=None):
    from concourse.bass_utils import run_bass_kernel_spmd

    feat_map = np.asarray(feat_map, np.float32)
    rois = np.asarray(rois, np.float32)
    offset = np.asarray(offset, np.float32)
    stride = int(stride)
    num_point = int(num_point)
    assert feat_map.shape == (B, C, H, W), feat_map.shape
    assert stride == STRIDE and num_point == NUM_POINT

    ids, S, in_maps = _host_prep(feat_map, rois, offset, num_point)
    nc = _get_program(S)
    res = run_bass_kernel_spmd(nc, in_maps, core_ids=list(range(NCORES)),
                               **(_collect.pop("spmd_kwargs", {}) if _collect else {}))
    if _collect is not None:
        _collect["res"] = res
    return _host_unshard(res.results, ids, S, num_point, rois.shape[0])
